# revision 6
# baseline (speedup 1.0000x reference)
"""Box filter (radius 8, window 17, zero-padded edges) over dims 2,3 of a
[8, 32, 512, 512] f32 tensor, on 8 Trainium2 NeuronCores.

Decomposition (validated vs the jax reference, rel err ~1e-6):
  - The per-axis filter with clipped windows is exactly multiplication by a
    banded ones matrix B (B[i,k] = 1 iff |i-k| <= 8), i.e. Z = B @ X @ B.
  - Column (free-dim) filter: ONE fused DVE `tensor_tensor_scan` per row-tile
    computes the sliding-window sum directly via the recurrence
        state[t] = (x[t] + state[t-1]) - x[t-17]
    over a zero-padded buffer (17 zeros in front, 8 behind), so scan output
    position t holds the window ending at t; the window *centered* at c is
    position c+8, read as a simple offset view.
  - Row (partition-dim) filter: one PE matmul per 112-row output tile with a
    host-built banded lhsT (input tiles carry an 8-row halo on each side, so
    one K<=128 matmul covers the whole band).

Sharding: data-parallel over batch (dim 0) -> 8 cores, one batch each.
"""

import os
import sys

import numpy as np

for _p in ("/opt/trn_rl_repo", "/root/.axon_site/_ro/trn_rl_repo"):
    if os.path.isdir(_p) and _p not in sys.path:
        sys.path.append(_p)

import concourse.bass as bass
import concourse.tile as tile
from concourse import bacc, mybir
from concourse.bass_utils import run_bass_kernel_spmd

R = 8
PADF = 2 * R + 1  # front zero pad (window width)
PADB = R          # back zero pad
H = W = 512
CH = 32
NCORES = 8

# Row-tile specs: (row_start, n_rows_loaded, use_first_B, out_rows, out_start).
# Output tiles are 112 rows; input tiles carry the +-8 halo (clipped at the
# image edges), so a single matmul covers the full 17-row band.
SPECS = [
    (0, 120, True, 112, 0),
    (104, 128, False, 112, 112),
    (216, 128, False, 112, 224),
    (328, 128, False, 112, 336),
    (440, 72, False, 64, 448),
]

_CACHE = {}


def _banded():
    # Bl[k, m] = 1 iff the input row at tile partition k (image row
    # 112*t - 8 + k) is inside the window of output row m (image row 112*t+m):
    # |(m + 8) - k| <= 8  <=>  m <= k <= m + 16.
    k = np.arange(128)[:, None]
    m = np.arange(112)[None, :]
    bl = ((m <= k) & (k <= m + 16)).astype(np.float32)
    # First tile starts at image row 0 (no left halo): partition k = image
    # row k, band |k - m| <= 8 — which is bl shifted down 8 partitions.
    blf = bl[8:128].copy()
    return bl, blf


def _build_program():
    if "nc" in _CACHE:
        return _CACHE["nc"]
    # Bacc (not raw Bass): its compile() legalizes sync waits — TRN2 allows
    # at most 1 wait per instruction; excess waits become standalone
    # EventSemaphore instructions (and matmul waits move to ldweights).
    nc = bacc.Bacc(debug=False)
    x = nc.dram_tensor("x", [CH, H, W], mybir.dt.float32, kind="ExternalInput")
    z = nc.dram_tensor("z", [CH, H, W], mybir.dt.float32, kind="ExternalOutput")
    bl = nc.dram_tensor("bl", [128, 112], mybir.dt.float32, kind="ExternalInput")
    blf = nc.dram_tensor("blf", [120, 112], mybir.dt.float32, kind="ExternalInput")
    xap, zap = x.ap(), z.ap()

    f32 = mybir.dt.float32
    XW = PADF + W + PADB  # 537

    with tile.TileContext(nc) as tc:
        with (
            tc.tile_pool(name="consts", bufs=1) as cpool,
            tc.tile_pool(name="xpad", bufs=4) as xpool,
            tc.tile_pool(name="ubuf", bufs=4) as upool,
            tc.tile_pool(name="obuf", bufs=4) as opool,
            tc.tile_pool(name="psum", bufs=4, space="PSUM") as ppool,
        ):
            blt = cpool.tile([128, 112], f32)
            nc.sync.dma_start(blt[:], bl.ap()[:, :])
            blft = cpool.tile([120, 112], f32)
            nc.sync.dma_start(blft[:], blf.ap()[:, :])

            for c in range(CH):
                for (r0, nr, first, m_out, o0) in SPECS:
                    xp = xpool.tile([128, XW], f32)
                    nc.vector.memset(xp[:, 0:PADF], 0.0)
                    nc.vector.memset(xp[:, PADF + W:], 0.0)
                    nc.sync.dma_start(
                        xp[0:nr, PADF:PADF + W], xap[c, r0:r0 + nr, :]
                    )
                    ub = upool.tile([128, W + PADB], f32)
                    nc.vector.tensor_tensor_scan(
                        out=ub[0:nr, :],
                        data0=xp[0:nr, PADF:],
                        data1=xp[0:nr, 0:W + PADB],
                        initial=0.0,
                        op0=mybir.AluOpType.add,
                        op1=mybir.AluOpType.subtract,
                    )
                    ps = ppool.tile([112, 512], f32)
                    lhsT = blft[0:nr, 0:m_out] if first else blt[0:nr, 0:m_out]
                    nc.tensor.matmul(
                        ps[0:m_out, :], lhsT, ub[0:nr, R:R + W],
                        start=True, stop=True,
                    )
                    ob = opool.tile([112, 512], f32)
                    nc.scalar.copy(ob[0:m_out, :], ps[0:m_out, :])
                    nc.sync.dma_start(zap[c, o0:o0 + m_out, :], ob[0:m_out, :])

    nc.compile()
    _CACHE["nc"] = nc
    return nc


def kernel(tensor: np.ndarray) -> np.ndarray:
    tensor = np.ascontiguousarray(np.asarray(tensor, dtype=np.float32))
    assert tensor.shape == (NCORES, CH, H, W)
    bl, blf = _banded()
    nc = _build_program()
    in_maps = [
        {"x": tensor[i], "bl": bl, "blf": blf} for i in range(NCORES)
    ]
    res = run_bass_kernel_spmd(nc, in_maps, core_ids=list(range(NCORES)))
    return np.stack([res.results[i]["z"] for i in range(NCORES)], axis=0)


# revision 7
# speedup vs baseline: 1.1814x; 1.1814x over previous
"""Box filter (radius 8, window 17, zero-padded edges) over dims 2,3 of a
[8, 32, 512, 512] f32 tensor, on 8 Trainium2 NeuronCores.

Decomposition (validated vs the jax reference, rel err ~1e-6):
  - The per-axis filter with clipped windows is exactly multiplication by a
    banded ones matrix B (B[i,k] = 1 iff |i-k| <= 8), i.e. Z = B @ X @ B.
  - Column (free-dim) filter: ONE fused DVE `tensor_tensor_scan` per row-tile
    computes the sliding-window sum directly via the recurrence
        state[t] = (x[t] + state[t-1]) - x[t-17]
    over a zero-padded buffer (17 zeros in front, 8 behind), so scan output
    position t holds the window ending at t; the window *centered* at c is
    position c+8, read as a simple offset view.
  - Row (partition-dim) filter: one PE matmul per 112-row output tile with a
    host-built banded lhsT (input tiles carry an 8-row halo on each side, so
    one K<=128 matmul covers the whole band).

Sharding: data-parallel over batch (dim 0) -> 8 cores, one batch each.
"""

import os
import sys

import numpy as np

for _p in ("/opt/trn_rl_repo", "/root/.axon_site/_ro/trn_rl_repo"):
    if os.path.isdir(_p) and _p not in sys.path:
        sys.path.append(_p)

import concourse.bass as bass
import concourse.tile as tile
from concourse import bacc, mybir
from concourse.bass_utils import run_bass_kernel_spmd

R = 8
PADF = 2 * R + 1  # front zero pad (window width)
PADB = R          # back zero pad
H = W = 512
CH = 32
NCORES = 8

# Row-tile specs: (row_start, n_rows_loaded, use_first_B, out_rows, out_start).
# Output tiles are 112 rows; input tiles carry the +-8 halo (clipped at the
# image edges), so a single matmul covers the full 17-row band.
SPECS = [
    (0, 120, True, 112, 0),
    (104, 128, False, 112, 112),
    (216, 128, False, 112, 224),
    (328, 128, False, 112, 336),
    (440, 72, False, 64, 448),
]

_CACHE = {}


def _banded():
    # Bl[k, m] = 1 iff the input row at tile partition k (image row
    # 112*t - 8 + k) is inside the window of output row m (image row 112*t+m):
    # |(m + 8) - k| <= 8  <=>  m <= k <= m + 16.
    k = np.arange(128)[:, None]
    m = np.arange(112)[None, :]
    bl = ((m <= k) & (k <= m + 16)).astype(np.float32)
    # First tile starts at image row 0 (no left halo): partition k = image
    # row k, band |k - m| <= 8 — which is bl shifted down 8 partitions.
    blf = bl[8:128].copy()
    return bl, blf


def _build_program():
    if "nc" in _CACHE:
        return _CACHE["nc"]
    # Bacc (not raw Bass): its compile() legalizes sync waits — TRN2 allows
    # at most 1 wait per instruction; excess waits become standalone
    # EventSemaphore instructions (and matmul waits move to ldweights).
    nc = bacc.Bacc(debug=False)
    x = nc.dram_tensor("x", [CH, H, W], mybir.dt.float32, kind="ExternalInput")
    z = nc.dram_tensor("z", [CH, H, W], mybir.dt.float32, kind="ExternalOutput")
    bl = nc.dram_tensor("bl", [128, 112], mybir.dt.float32, kind="ExternalInput")
    blf = nc.dram_tensor("blf", [120, 112], mybir.dt.float32, kind="ExternalInput")
    xap, zap = x.ap(), z.ap()

    f32 = mybir.dt.float32
    XW = PADF + W + PADB  # 537
    NXBUF = 6

    with tile.TileContext(nc) as tc:
        with (
            tc.tile_pool(name="consts", bufs=1) as cpool,
            tc.tile_pool(name="ubuf", bufs=6) as upool,
            tc.tile_pool(name="obuf", bufs=6) as opool,
            tc.tile_pool(name="psum", bufs=8, space="PSUM") as ppool,
        ):
            blt = cpool.tile([128, 112], f32)
            nc.sync.dma_start(blt[:], bl.ap()[:, :])
            blft = cpool.tile([120, 112], f32)
            nc.sync.dma_start(blft[:], blf.ap()[:, :])

            # Static input buffers (manual ring): their zero pads are written
            # once here and never touched again — the loads below only write
            # the data columns, so no per-iteration memset is needed.
            xbufs = [
                nc.alloc_sbuf_tensor(f"xpad{i}", [128, XW], f32).ap()
                for i in range(NXBUF)
            ]
            for xb in xbufs:
                nc.vector.memset(xb[:, 0:PADF], 0.0)
                nc.vector.memset(xb[:, PADF + W:], 0.0)

            it = 0
            for c in range(CH):
                for (r0, nr, first, m_out, o0) in SPECS:
                    xp = xbufs[it % NXBUF]
                    it += 1
                    nc.sync.dma_start(
                        xp[0:nr, PADF:PADF + W], xap[c, r0:r0 + nr, :]
                    )
                    ub = upool.tile([128, W + PADB], f32)
                    nc.vector.tensor_tensor_scan(
                        out=ub[0:nr, :],
                        data0=xp[0:nr, PADF:],
                        data1=xp[0:nr, 0:W + PADB],
                        initial=0.0,
                        op0=mybir.AluOpType.add,
                        op1=mybir.AluOpType.subtract,
                    )
                    ps = ppool.tile([112, 512], f32)
                    lhsT = blft[0:nr, 0:m_out] if first else blt[0:nr, 0:m_out]
                    nc.tensor.matmul(
                        ps[0:m_out, :], lhsT, ub[0:nr, R:R + W],
                        start=True, stop=True,
                    )
                    ob = opool.tile([112, 512], f32)
                    nc.scalar.copy(ob[0:m_out, :], ps[0:m_out, :])
                    nc.sync.dma_start(zap[c, o0:o0 + m_out, :], ob[0:m_out, :])

    nc.compile()
    _CACHE["nc"] = nc
    return nc


def kernel(tensor: np.ndarray) -> np.ndarray:
    tensor = np.ascontiguousarray(np.asarray(tensor, dtype=np.float32))
    assert tensor.shape == (NCORES, CH, H, W)
    bl, blf = _banded()
    nc = _build_program()
    in_maps = [
        {"x": tensor[i], "bl": bl, "blf": blf} for i in range(NCORES)
    ]
    res = run_bass_kernel_spmd(nc, in_maps, core_ids=list(range(NCORES)))
    return np.stack([res.results[i]["z"] for i in range(NCORES)], axis=0)


# revision 8
# speedup vs baseline: 1.2157x; 1.0291x over previous
"""Box filter (radius 8, window 17, zero-padded edges) over dims 2,3 of a
[8, 32, 512, 512] f32 tensor, on 8 Trainium2 NeuronCores.

Decomposition (validated vs the jax reference, rel err ~1e-6):
  - The per-axis filter with clipped windows is exactly multiplication by a
    banded ones matrix B (B[i,k] = 1 iff |i-k| <= 8), i.e. Z = B @ X @ B.
  - Column (free-dim) filter: ONE fused DVE `tensor_tensor_scan` per row-tile
    computes the sliding-window sum directly via the recurrence
        state[t] = (x[t] + state[t-1]) - x[t-17]
    over a zero-padded buffer (17 zeros in front, 8 behind), so scan output
    position t holds the window ending at t; the window *centered* at c is
    position c+8, read as a simple offset view.
  - Row (partition-dim) filter: one PE matmul per 112-row output tile with a
    host-built banded lhsT (input tiles carry an 8-row halo on each side, so
    one K<=128 matmul covers the whole band).

Sharding: data-parallel over batch (dim 0) -> 8 cores, one batch each.
"""

import os
import sys

import numpy as np

for _p in ("/opt/trn_rl_repo", "/root/.axon_site/_ro/trn_rl_repo"):
    if os.path.isdir(_p) and _p not in sys.path:
        sys.path.append(_p)

import concourse.bass as bass
import concourse.tile as tile
from concourse import bacc, mybir
from concourse.bass_utils import run_bass_kernel_spmd

R = 8
PADF = 2 * R + 1  # front zero pad (window width)
PADB = R          # back zero pad
H = W = 512
CH = 32
NCORES = 8

# Row-tile specs: (row_start, n_rows_loaded, use_first_B, out_rows, out_start).
# Output tiles are 112 rows; input tiles carry the +-8 halo (clipped at the
# image edges), so a single matmul covers the full 17-row band.
SPECS = [
    (0, 120, True, 112, 0),
    (104, 128, False, 112, 112),
    (216, 128, False, 112, 224),
    (328, 128, False, 112, 336),
    (440, 72, False, 64, 448),
]

_CACHE = {}


def _banded():
    # Bl[k, m] = 1 iff the input row at tile partition k (image row
    # 112*t - 8 + k) is inside the window of output row m (image row 112*t+m):
    # |(m + 8) - k| <= 8  <=>  m <= k <= m + 16.
    k = np.arange(128)[:, None]
    m = np.arange(112)[None, :]
    bl = ((m <= k) & (k <= m + 16)).astype(np.float32)
    # First tile starts at image row 0 (no left halo): partition k = image
    # row k, band |k - m| <= 8 — which is bl shifted down 8 partitions.
    blf = bl[8:128].copy()
    return bl, blf


def _build_program():
    if "nc" in _CACHE:
        return _CACHE["nc"]
    # Bacc (not raw Bass): its compile() legalizes sync waits — TRN2 allows
    # at most 1 wait per instruction; excess waits become standalone
    # EventSemaphore instructions (and matmul waits move to ldweights).
    nc = bacc.Bacc(debug=False)
    x = nc.dram_tensor("x", [CH, H, W], mybir.dt.float32, kind="ExternalInput")
    z = nc.dram_tensor("z", [CH, H, W], mybir.dt.float32, kind="ExternalOutput")
    bl = nc.dram_tensor("bl", [128, 112], mybir.dt.float32, kind="ExternalInput")
    blf = nc.dram_tensor("blf", [120, 112], mybir.dt.float32, kind="ExternalInput")
    xap, zap = x.ap(), z.ap()

    f32 = mybir.dt.float32
    XW = PADF + W + PADB  # 537
    NXBUF = 10

    with tile.TileContext(nc) as tc:
        with (
            tc.tile_pool(name="consts", bufs=1) as cpool,
            tc.tile_pool(name="ubuf", bufs=8) as upool,
            tc.tile_pool(name="obuf", bufs=10) as opool,
            tc.tile_pool(name="psum", bufs=8, space="PSUM") as ppool,
        ):
            blt = cpool.tile([128, 112], f32)
            nc.sync.dma_start(blt[:], bl.ap()[:, :])
            blft = cpool.tile([120, 112], f32)
            nc.sync.dma_start(blft[:], blf.ap()[:, :])

            # Static input buffers (manual ring): their zero pads are written
            # once here and never touched again — the loads below only write
            # the data columns, so no per-iteration memset is needed.
            xbufs = [
                nc.alloc_sbuf_tensor(f"xpad{i}", [128, XW], f32).ap()
                for i in range(NXBUF)
            ]
            for xb in xbufs:
                nc.vector.memset(xb[:, 0:PADF], 0.0)
                nc.vector.memset(xb[:, PADF + W:], 0.0)

            it = 0
            for c in range(CH):
                for (r0, nr, first, m_out, o0) in SPECS:
                    xp = xbufs[it % NXBUF]
                    it += 1
                    nc.sync.dma_start(
                        xp[0:nr, PADF:PADF + W], xap[c, r0:r0 + nr, :]
                    )
                    ub = upool.tile([128, W + PADB], f32)
                    nc.vector.tensor_tensor_scan(
                        out=ub[0:nr, :],
                        data0=xp[0:nr, PADF:],
                        data1=xp[0:nr, 0:W + PADB],
                        initial=0.0,
                        op0=mybir.AluOpType.add,
                        op1=mybir.AluOpType.subtract,
                    )
                    ps = ppool.tile([112, 512], f32)
                    lhsT = blft[0:nr, 0:m_out] if first else blt[0:nr, 0:m_out]
                    nc.tensor.matmul(
                        ps[0:m_out, :], lhsT, ub[0:nr, R:R + W],
                        start=True, stop=True,
                    )
                    ob = opool.tile([112, 512], f32)
                    nc.scalar.copy(ob[0:m_out, :], ps[0:m_out, :])
                    nc.sync.dma_start(zap[c, o0:o0 + m_out, :], ob[0:m_out, :])

    nc.compile()
    _CACHE["nc"] = nc
    return nc


def kernel(tensor: np.ndarray) -> np.ndarray:
    tensor = np.ascontiguousarray(np.asarray(tensor, dtype=np.float32))
    assert tensor.shape == (NCORES, CH, H, W)
    bl, blf = _banded()
    nc = _build_program()
    in_maps = [
        {"x": tensor[i], "bl": bl, "blf": blf} for i in range(NCORES)
    ]
    res = run_bass_kernel_spmd(nc, in_maps, core_ids=list(range(NCORES)))
    return np.stack([res.results[i]["z"] for i in range(NCORES)], axis=0)


# revision 15
# speedup vs baseline: 1.5300x; 1.2585x over previous
"""Box filter (radius 8, window 17, zero-padded edges) over dims 2,3 of a
[8, 32, 512, 512] f32 tensor, on 8 Trainium2 NeuronCores.

Decomposition (validated vs the jax reference, rel err ~1e-6):
  - The per-axis filter with clipped windows is exactly multiplication by a
    banded ones matrix B (B[i,k] = 1 iff |i-k| <= 8), i.e. Z = B @ X @ B.
  - Column (free-dim) filter: ONE fused DVE `tensor_tensor_scan` per row-tile
    computes the sliding-window sum directly via the recurrence
        state[t] = (x[t] + state[t-1]) - x[t-17]
    over a zero-padded buffer (17 zeros in front, 8 behind), so scan output
    position t holds the window ending at t; the window *centered* at c is
    position c+8, read as a simple offset view.
  - Row (partition-dim) filter: one PE matmul per 112-row output tile with a
    host-built banded lhsT (input tiles carry an 8-row halo on each side, so
    one K<=128 matmul covers the whole band).

Sharding: data-parallel over batch (dim 0) -> 8 cores, one batch each.
"""

import os
import sys

import numpy as np

for _p in ("/opt/trn_rl_repo", "/root/.axon_site/_ro/trn_rl_repo"):
    if os.path.isdir(_p) and _p not in sys.path:
        sys.path.append(_p)

import concourse.bass as bass
import concourse.tile as tile
from concourse import bacc, mybir
from concourse.bass_utils import run_bass_kernel_spmd

R = 8
PADF = 2 * R + 1  # front zero pad (window width)
PADB = R          # back zero pad
H = W = 512
CH = 32
NCORES = 8

# Row-tile specs: (row_start, n_rows_loaded, use_first_B, out_rows, out_start).
# Output tiles are 112 rows; input tiles carry the +-8 halo (clipped at the
# image edges), so a single matmul covers the full 17-row band.
SPECS = [
    (0, 120, True, 112, 0),
    (104, 128, False, 112, 112),
    (216, 128, False, 112, 224),
    (328, 128, False, 112, 336),
    (440, 72, False, 64, 448),
]

_CACHE = {}


def _banded():
    # Bl[k, m] = 1 iff the input row at tile partition k (image row
    # 112*t - 8 + k) is inside the window of output row m (image row 112*t+m):
    # |(m + 8) - k| <= 8  <=>  m <= k <= m + 16.
    k = np.arange(128)[:, None]
    m = np.arange(112)[None, :]
    bl = ((m <= k) & (k <= m + 16)).astype(np.float32)
    # First tile starts at image row 0 (no left halo): partition k = image
    # row k, band |k - m| <= 8 — which is bl shifted down 8 partitions.
    blf = bl[8:128].copy()
    return bl, blf


USE_F32R = os.environ.get("BOX_F32R", "0") == "1"


def _build_program():
    if "nc" in _CACHE:
        return _CACHE["nc"]
    # Bacc (not raw Bass): its compile() legalizes sync waits — TRN2 allows
    # at most 1 wait per instruction; excess waits become standalone
    # EventSemaphore instructions (and matmul waits move to ldweights).
    nc = bacc.Bacc(debug=False)
    # float32r (tf32) matmul operands run the PE at 1 cycle/row instead of
    # fp32's 2x half-speed passes; the walrus verifier requires fp32r
    # operands to be produced as fp32r, so the B constants and the scan
    # output use the dtype end-to-end (same 4-byte storage as fp32).
    mm_dt = mybir.dt.float32r if USE_F32R else mybir.dt.float32
    x = nc.dram_tensor("x", [CH, H, W], mybir.dt.float32, kind="ExternalInput")
    z = nc.dram_tensor("z", [CH, H, W], mybir.dt.float32, kind="ExternalOutput")
    bl = nc.dram_tensor("bl", [128, 112], mm_dt, kind="ExternalInput")
    blf = nc.dram_tensor("blf", [120, 112], mm_dt, kind="ExternalInput")
    xap, zap = x.ap(), z.ap()

    f32 = mybir.dt.float32
    XW = PADF + W + PADB  # 537
    NXBUF = 10

    with tile.TileContext(nc) as tc:
        with (
            tc.tile_pool(name="consts", bufs=1) as cpool,
            tc.tile_pool(name="ubuf", bufs=8) as upool,
            tc.tile_pool(name="obuf", bufs=10) as opool,
            tc.tile_pool(name="psum", bufs=8, space="PSUM") as ppool,
        ):
            blt = cpool.tile([128, 112], mm_dt)
            nc.sync.dma_start(blt[:], bl.ap()[:, :])
            blft = cpool.tile([120, 112], mm_dt)
            nc.sync.dma_start(blft[:], blf.ap()[:, :])

            # Static input buffers (manual ring): their zero pads are written
            # once here and never touched again — the loads below only write
            # the data columns, so no per-iteration memset is needed.
            xbufs = [
                nc.alloc_sbuf_tensor(f"xpad{i}", [128, XW], f32).ap()
                for i in range(NXBUF)
            ]
            for xb in xbufs:
                nc.vector.memset(xb[:, 0:PADF], 0.0)
                nc.vector.memset(xb[:, PADF + W:], 0.0)

            it = 0
            for c in range(CH):
                for (r0, nr, first, m_out, o0) in SPECS:
                    xp = xbufs[it % NXBUF]
                    it += 1
                    nc.sync.dma_start(
                        xp[0:nr, PADF:PADF + W], xap[c, r0:r0 + nr, :]
                    )
                    ub = upool.tile([128, W + PADB], mm_dt)
                    nc.vector.tensor_tensor_scan(
                        out=ub[0:nr, :],
                        data0=xp[0:nr, PADF:],
                        data1=xp[0:nr, 0:W + PADB],
                        initial=0.0,
                        op0=mybir.AluOpType.add,
                        op1=mybir.AluOpType.subtract,
                    )
                    ps = ppool.tile([112, 512], f32)
                    lhsT = blft[0:nr, 0:m_out] if first else blt[0:nr, 0:m_out]
                    nc.tensor.matmul(
                        ps[0:m_out, :], lhsT, ub[0:nr, R:R + W],
                        start=True, stop=True,
                    )
                    ob = opool.tile([112, 512], f32)
                    nc.scalar.copy(ob[0:m_out, :], ps[0:m_out, :])
                    # Store on the scalar HWDGE ring: it directly follows the
                    # copy on the same engine (program order, no sem wait) and
                    # keeps the sync ring free for loads.
                    nc.scalar.dma_start(zap[c, o0:o0 + m_out, :], ob[0:m_out, :])

    nc.compile()
    _CACHE["nc"] = nc
    return nc


def kernel(tensor: np.ndarray) -> np.ndarray:
    tensor = np.ascontiguousarray(np.asarray(tensor, dtype=np.float32))
    assert tensor.shape == (NCORES, CH, H, W)
    bl, blf = _banded()
    nc = _build_program()
    in_maps = [
        {"x": tensor[i], "bl": bl, "blf": blf} for i in range(NCORES)
    ]
    res = run_bass_kernel_spmd(nc, in_maps, core_ids=list(range(NCORES)))
    return np.stack([res.results[i]["z"] for i in range(NCORES)], axis=0)


# revision 16
# speedup vs baseline: 1.5511x; 1.0138x over previous
"""Box filter (radius 8, window 17, zero-padded edges) over dims 2,3 of a
[8, 32, 512, 512] f32 tensor, on 8 Trainium2 NeuronCores.

Decomposition (validated vs the jax reference, rel err ~1e-6):
  - The per-axis filter with clipped windows is exactly multiplication by a
    banded ones matrix B (B[i,k] = 1 iff |i-k| <= 8), i.e. Z = B @ X @ B.
  - Column (free-dim) filter: ONE fused DVE `tensor_tensor_scan` per row-tile
    computes the sliding-window sum directly via the recurrence
        state[t] = (x[t] + state[t-1]) - x[t-17]
    over a zero-padded buffer (17 zeros in front, 8 behind), so scan output
    position t holds the window ending at t; the window *centered* at c is
    position c+8, read as a simple offset view.
  - Row (partition-dim) filter: one PE matmul per 112-row output tile with a
    host-built banded lhsT (input tiles carry an 8-row halo on each side, so
    one K<=128 matmul covers the whole band).

Sharding: data-parallel over batch (dim 0) -> 8 cores, one batch each.
"""

import os
import sys

import numpy as np

for _p in ("/opt/trn_rl_repo", "/root/.axon_site/_ro/trn_rl_repo"):
    if os.path.isdir(_p) and _p not in sys.path:
        sys.path.append(_p)

import concourse.bass as bass
import concourse.tile as tile
from concourse import bacc, mybir
from concourse.bass_utils import run_bass_kernel_spmd

R = 8
PADF = 2 * R + 1  # front zero pad (window width)
PADB = R          # back zero pad
H = W = 512
CH = 32
NCORES = 8

# Row-tile specs: (row_start, n_rows_loaded, use_first_B, out_rows, out_start).
# Output tiles are 112 rows; input tiles carry the +-8 halo (clipped at the
# image edges), so a single matmul covers the full 17-row band.
SPECS = [
    (0, 120, True, 112, 0),
    (104, 128, False, 112, 112),
    (216, 128, False, 112, 224),
    (328, 128, False, 112, 336),
    (440, 72, False, 64, 448),
]

_CACHE = {}


def _banded():
    # Bl[k, m] = 1 iff the input row at tile partition k (image row
    # 112*t - 8 + k) is inside the window of output row m (image row 112*t+m):
    # |(m + 8) - k| <= 8  <=>  m <= k <= m + 16.
    k = np.arange(128)[:, None]
    m = np.arange(112)[None, :]
    bl = ((m <= k) & (k <= m + 16)).astype(np.float32)
    # First tile starts at image row 0 (no left halo): partition k = image
    # row k, band |k - m| <= 8 — which is bl shifted down 8 partitions.
    blf = bl[8:128].copy()
    return bl, blf


USE_F32R = os.environ.get("BOX_F32R", "0") == "1"
GPS_SCAN = os.environ.get("BOX_GPS_SCAN", "0") == "1"


def _build_program():
    if "nc" in _CACHE:
        return _CACHE["nc"]
    # Bacc (not raw Bass): its compile() legalizes sync waits — TRN2 allows
    # at most 1 wait per instruction; excess waits become standalone
    # EventSemaphore instructions (and matmul waits move to ldweights).
    nc = bacc.Bacc(debug=False)
    # float32r (tf32) matmul operands run the PE at 1 cycle/row instead of
    # fp32's 2x half-speed passes; the walrus verifier requires fp32r
    # operands to be produced as fp32r, so the B constants and the scan
    # output use the dtype end-to-end (same 4-byte storage as fp32).
    mm_dt = mybir.dt.float32r if USE_F32R else mybir.dt.float32
    x = nc.dram_tensor("x", [CH, H, W], mybir.dt.float32, kind="ExternalInput")
    z = nc.dram_tensor("z", [CH, H, W], mybir.dt.float32, kind="ExternalOutput")
    bl = nc.dram_tensor("bl", [128, 112], mm_dt, kind="ExternalInput")
    blf = nc.dram_tensor("blf", [120, 112], mm_dt, kind="ExternalInput")
    xap, zap = x.ap(), z.ap()

    f32 = mybir.dt.float32
    XW = PADF + W + PADB  # 537
    NXBUF = 12

    with tile.TileContext(nc) as tc:
        with (
            tc.tile_pool(name="consts", bufs=1) as cpool,
            tc.tile_pool(name="ubuf", bufs=10) as upool,
            tc.tile_pool(name="obuf", bufs=12) as opool,
            tc.tile_pool(name="psum", bufs=8, space="PSUM") as ppool,
        ):
            blt = cpool.tile([128, 112], mm_dt)
            nc.sync.dma_start(blt[:], bl.ap()[:, :])
            blft = cpool.tile([120, 112], mm_dt)
            nc.sync.dma_start(blft[:], blf.ap()[:, :])

            # Static input buffers (manual ring): their zero pads are written
            # once here and never touched again — the loads below only write
            # the data columns, so no per-iteration memset is needed.
            xbufs = [
                nc.alloc_sbuf_tensor(f"xpad{i}", [128, XW], f32).ap()
                for i in range(NXBUF)
            ]
            for xb in xbufs:
                nc.vector.memset(xb[:, 0:PADF], 0.0)
                nc.vector.memset(xb[:, PADF + W:], 0.0)

            it = 0
            for c in range(CH):
                for (r0, nr, first, m_out, o0) in SPECS:
                    xp = xbufs[it % NXBUF]
                    it += 1
                    nc.sync.dma_start(
                        xp[0:nr, PADF:PADF + W], xap[c, r0:r0 + nr, :]
                    )
                    ub = upool.tile([128, W + PADB], mm_dt)
                    scan_eng = (
                        nc.gpsimd if (GPS_SCAN and m_out == 64) else nc.vector
                    )
                    scan_eng.tensor_tensor_scan(
                        out=ub[0:nr, :],
                        data0=xp[0:nr, PADF:],
                        data1=xp[0:nr, 0:W + PADB],
                        initial=0.0,
                        op0=mybir.AluOpType.add,
                        op1=mybir.AluOpType.subtract,
                    )
                    ps = ppool.tile([112, 512], f32)
                    lhsT = blft[0:nr, 0:m_out] if first else blt[0:nr, 0:m_out]
                    nc.tensor.matmul(
                        ps[0:m_out, :], lhsT, ub[0:nr, R:R + W],
                        start=True, stop=True,
                    )
                    ob = opool.tile([112, 512], f32)
                    nc.scalar.copy(ob[0:m_out, :], ps[0:m_out, :])
                    # Store on the scalar HWDGE ring: it directly follows the
                    # copy on the same engine (program order, no sem wait) and
                    # keeps the sync ring free for loads.
                    nc.scalar.dma_start(zap[c, o0:o0 + m_out, :], ob[0:m_out, :])

    nc.compile()
    _CACHE["nc"] = nc
    return nc


def kernel(tensor: np.ndarray) -> np.ndarray:
    tensor = np.ascontiguousarray(np.asarray(tensor, dtype=np.float32))
    assert tensor.shape == (NCORES, CH, H, W)
    bl, blf = _banded()
    nc = _build_program()
    in_maps = [
        {"x": tensor[i], "bl": bl, "blf": blf} for i in range(NCORES)
    ]
    res = run_bass_kernel_spmd(nc, in_maps, core_ids=list(range(NCORES)))
    return np.stack([res.results[i]["z"] for i in range(NCORES)], axis=0)
